# revision 7
# baseline (speedup 1.0000x reference)
"""Trainium2 Bass kernel for the CCSA (criss-cross self-attention) module.

The reference adds +INF_VAL (3.4e38, finite) on the H-axis diagonal of the
energy tensor before a joint softmax over the concatenated H+W axis.  In
float32 that makes the softmax an EXACT one-hot on the diagonal entry
(exp(small - 3.4e38) underflows to 0, exp(0) = 1), so att_h == I and
att_w == 0 identically, and the module collapses (bit-exactly, verified
against the jax reference) to:

    out = gamma * (x @ Wh + bh) + x

i.e. a residual 1x1 convolution.  The kernel below computes exactly that:
data-parallel over batch (one image per NeuronCore), per-core GEMM
[16384, 256] @ [256, 256] with the residual add fused in the epilogue.

Per-core pipeline (128-pixel chunks):
  - DMA a group of 2048 pixels [128, 16, 256] (p-major -> 16 KiB contiguous
    DRAM runs per partition)
  - PE-transpose each chunk's two 128-channel halves (C must sit on the
    partition axis for the contraction)
  - copy PSUM -> SBUF (ACT + DVE)
  - 2 accumulating matmuls (fp32r, moving operand Whg [128, 256])
  - DVE epilogue: out = psum + x (gamma folded into the weights host-side)
  - DMA the group back out
"""

import numpy as np

import concourse.bacc as bacc
import concourse.tile as tile
from concourse import mybir
from concourse import bass_utils

# Shapes fixed by the problem: x is [8, 128, 128, 256] float32.
NCORES = 8
P = 128            # SBUF partitions == pixels per chunk
C = 256            # channels
PIX = 128 * 128    # pixels per image
G = 16             # chunks per DMA group (2048 pixels, 2 MiB per transfer)
NGRP = PIX // (P * G)

F32 = mybir.dt.float32
F32R = mybir.dt.float32r

_last_results = None  # test.py reads exec_time_ns from here
_last_nc = None       # test.py runs TimelineSim on this


def _build(has_bias: bool):
    nc = bacc.Bacc("TRN2", target_bir_lowering=False, debug=False,
                   num_devices=NCORES)
    x_d = nc.dram_tensor("x", [PIX, C], F32, kind="ExternalInput")
    whg_d = nc.dram_tensor("whg", [C, C], F32R, kind="ExternalInput")
    idn_d = nc.dram_tensor("idn", [P, P], F32, kind="ExternalInput")
    if has_bias:
        ones_d = nc.dram_tensor("ones", [1, P], F32R, kind="ExternalInput")
        bhg_d = nc.dram_tensor("bhg", [1, C], F32R, kind="ExternalInput")
    out_d = nc.dram_tensor("out", [PIX, C], F32, kind="ExternalOutput")

    # pixel index = n*(P*G) + p*G + g: each partition p owns G consecutive
    # pixels, so its DRAM run is G*C*4 = 16 KiB contiguous.
    xv = x_d.ap().rearrange("(n p g) c -> n p g c", n=NGRP, p=P, g=G)
    ov = out_d.ap().rearrange("(n p g) c -> n p g c", n=NGRP, p=P, g=G)

    with tile.TileContext(nc) as tc:
        with (
            tc.tile_pool(name="const", bufs=1) as cpool,
            tc.tile_pool(name="xin", bufs=2) as xin_pool,
            tc.tile_pool(name="xout", bufs=2) as xout_pool,
            tc.tile_pool(name="xt", bufs=3) as xt_pool,
            tc.tile_pool(name="pst", bufs=2, space="PSUM") as pst_pool,
            tc.tile_pool(name="pso", bufs=2, space="PSUM") as pso_pool,
        ):
            whg_sb = cpool.tile([P, 2, C], F32R)
            nc.sync.dma_start(whg_sb[:],
                              whg_d.ap().rearrange("(k p) c -> p k c", k=2))
            idn_sb = cpool.tile([P, P], F32)
            nc.sync.dma_start(idn_sb[:], idn_d.ap())
            if has_bias:
                ones_sb = cpool.tile([1, P], F32R)
                nc.sync.dma_start(ones_sb[:], ones_d.ap())
                bhg_sb = cpool.tile([1, C], F32R)
                nc.sync.dma_start(bhg_sb[:], bhg_d.ap())

            for n in range(NGRP):
                x_sb = xin_pool.tile([P, G, C], F32, tag="xin")
                nc.sync.dma_start(x_sb[:], xv[n])
                o_sb = xout_pool.tile([P, G, C], F32, tag="xout")
                for g in range(G):
                    pst0 = pst_pool.tile([P, P], F32, tag="pst0")
                    pst1 = pst_pool.tile([P, P], F32, tag="pst1")
                    nc.tensor.transpose(pst0[:], x_sb[:, g, 0:P], idn_sb[:])
                    nc.tensor.transpose(pst1[:], x_sb[:, g, P:C], idn_sb[:])
                    xt0 = xt_pool.tile([P, P], F32R, tag="xt0")
                    xt1 = xt_pool.tile([P, P], F32R, tag="xt1")
                    nc.scalar.copy(xt0[:], pst0[:])
                    nc.vector.tensor_copy(xt1[:], pst1[:])
                    pso = pso_pool.tile([P, C], F32, tag="pso")
                    nc.tensor.matmul(pso[:], xt0[:], whg_sb[:, 0, :],
                                     start=True, stop=False)
                    nc.tensor.matmul(pso[:], xt1[:], whg_sb[:, 1, :],
                                     start=False, stop=not has_bias)
                    if has_bias:
                        nc.tensor.matmul(pso[:], ones_sb[:], bhg_sb[:],
                                         start=False, stop=True)
                    nc.vector.tensor_add(o_sb[:, g, :], pso[:], x_sb[:, g, :])
                nc.sync.dma_start(ov[n], o_sb[:])
    nc.compile()
    return nc


def kernel(x, Wf, bf, Wg, bg, Wh, bh, gamma):
    global _last_results, _last_nc
    x = np.asarray(x, dtype=np.float32)
    Wh = np.asarray(Wh, dtype=np.float32)
    bh = np.asarray(bh, dtype=np.float32)
    gam = np.float32(np.asarray(gamma))
    B, H, W, Cc = x.shape
    assert (B, H * W, Cc) == (NCORES, PIX, C), (B, H, W, Cc)

    whg = np.ascontiguousarray(gam * Wh, dtype=np.float32)
    bhg = (gam * bh).astype(np.float32)
    has_bias = bool(np.any(bhg != 0))

    nc = _build(has_bias)
    _last_nc = nc
    idn = np.eye(P, dtype=np.float32)
    xf = np.ascontiguousarray(x.reshape(B, PIX, Cc))
    in_maps = []
    for b in range(B):
        m = {"x": xf[b], "whg": whg, "idn": idn}
        if has_bias:
            m["ones"] = np.ones((1, P), np.float32)
            m["bhg"] = np.ascontiguousarray(bhg.reshape(1, C))
        in_maps.append(m)

    res = bass_utils.run_bass_kernel_spmd(nc, in_maps,
                                          core_ids=list(range(NCORES)))
    _last_results = res
    out = np.stack([res.results[b]["out"] for b in range(B)], axis=0)
    return out.reshape(B, H, W, Cc)


# revision 8
# speedup vs baseline: 1.2681x; 1.2681x over previous
"""Trainium2 Bass kernel for the CCSA (criss-cross self-attention) module.

The reference adds +INF_VAL (3.4e38, finite) on the H-axis diagonal of the
energy tensor before a joint softmax over the concatenated H+W axis.  In
float32 that makes the softmax an EXACT one-hot on the diagonal entry
(exp(small - 3.4e38) underflows to 0, exp(0) = 1), so att_h == I and
att_w == 0 identically, and the module collapses (bit-exactly, verified
against the jax reference) to:

    out = gamma * (x @ Wh + bh) + x

i.e. a residual 1x1 convolution.  The kernel below computes exactly that:
data-parallel over batch (one image per NeuronCore), per-core GEMM
[16384, 256] @ [256, 256] with the residual add fused in the epilogue.

Per-core pipeline (128-pixel chunks):
  - DMA a group of 2048 pixels [128, 16, 256] (p-major -> 16 KiB contiguous
    DRAM runs per partition)
  - PE-transpose each chunk's two 128-channel halves (C must sit on the
    partition axis for the contraction)
  - copy PSUM -> SBUF (ACT + DVE)
  - 2 accumulating matmuls (fp32r, moving operand Whg [128, 256])
  - DVE epilogue: out = psum + x (gamma folded into the weights host-side)
  - DMA the group back out
"""

import numpy as np

import concourse.bacc as bacc
import concourse.tile as tile
from concourse import mybir
from concourse import bass_utils

# Shapes fixed by the problem: x is [8, 128, 128, 256] float32.
NCORES = 8
P = 128            # SBUF partitions == pixels per chunk
C = 256            # channels
PIX = 128 * 128    # pixels per image
G = 16             # chunks per DMA group (2048 pixels, 2 MiB per transfer)
NGRP = PIX // (P * G)

F32 = mybir.dt.float32
F32R = mybir.dt.float32r

_last_results = None  # test.py reads exec_time_ns from here
_last_nc = None       # test.py runs TimelineSim on this


def _build(has_bias: bool):
    nc = bacc.Bacc("TRN2", target_bir_lowering=False, debug=False,
                   num_devices=NCORES)
    x_d = nc.dram_tensor("x", [PIX, C], F32, kind="ExternalInput")
    whg_d = nc.dram_tensor("whg", [C, C], F32R, kind="ExternalInput")
    idn_d = nc.dram_tensor("idn", [P, P], F32, kind="ExternalInput")
    if has_bias:
        ones_d = nc.dram_tensor("ones", [1, P], F32R, kind="ExternalInput")
        bhg_d = nc.dram_tensor("bhg", [1, C], F32R, kind="ExternalInput")
    out_d = nc.dram_tensor("out", [PIX, C], F32, kind="ExternalOutput")

    # pixel index = n*(P*G) + p*G + g: each partition p owns G consecutive
    # pixels, so its DRAM run is G*C*4 = 16 KiB contiguous.
    xv = x_d.ap().rearrange("(n p g) c -> n p g c", n=NGRP, p=P, g=G)
    ov = out_d.ap().rearrange("(n p g) c -> n p g c", n=NGRP, p=P, g=G)

    LS = 2   # load pieces per group (1 MiB each)
    SS = 8   # store pieces per group (512 KiB each)
    with tile.TileContext(nc) as tc:
        with (
            tc.tile_pool(name="const", bufs=1) as cpool,
            tc.tile_pool(name="xin", bufs=3) as xin_pool,
            tc.tile_pool(name="xout", bufs=3) as xout_pool,
            tc.tile_pool(name="xt", bufs=3) as xt_pool,
            tc.tile_pool(name="pst", bufs=3, space="PSUM") as pst_pool,
            tc.tile_pool(name="pso", bufs=2, space="PSUM") as pso_pool,
        ):
            whg_sb = cpool.tile([P, 2, C], F32R)
            nc.sync.dma_start(whg_sb[:],
                              whg_d.ap().rearrange("(k p) c -> p k c", k=2))
            idn_sb = cpool.tile([P, P], F32)
            nc.sync.dma_start(idn_sb[:], idn_d.ap())
            if has_bias:
                ones_sb = cpool.tile([1, P], F32R)
                nc.sync.dma_start(ones_sb[:], ones_d.ap())
                bhg_sb = cpool.tile([1, C], F32R)
                nc.sync.dma_start(bhg_sb[:], bhg_d.ap())

            for n in range(NGRP):
                x_sb = xin_pool.tile([P, G, C], F32, tag="xin")
                gl = G // LS
                for s in range(LS):
                    nc.sync.dma_start(x_sb[:, s * gl:(s + 1) * gl, :],
                                      xv[n, :, s * gl:(s + 1) * gl, :])
                o_sb = xout_pool.tile([P, G, C], F32, tag="xout")
                for g in range(G):
                    pst = pst_pool.tile([P, C], F32, tag="pst")
                    nc.tensor.transpose(pst[:, 0:P], x_sb[:, g, 0:P], idn_sb[:])
                    nc.tensor.transpose(pst[:, P:C], x_sb[:, g, P:C], idn_sb[:])
                    xt = xt_pool.tile([P, C], F32R, tag="xt")
                    nc.scalar.copy(xt[:], pst[:])
                    pso = pso_pool.tile([P, C], F32, tag="pso")
                    nc.tensor.matmul(pso[:], xt[:, 0:P], whg_sb[:, 0, :],
                                     start=True, stop=False)
                    nc.tensor.matmul(pso[:], xt[:, P:C], whg_sb[:, 1, :],
                                     start=False, stop=not has_bias)
                    if has_bias:
                        nc.tensor.matmul(pso[:], ones_sb[:], bhg_sb[:],
                                         start=False, stop=True)
                    nc.vector.tensor_add(o_sb[:, g, :], pso[:], x_sb[:, g, :])
                gs = G // SS
                for s in range(SS):
                    nc.sync.dma_start(ov[n, :, s * gs:(s + 1) * gs, :],
                                      o_sb[:, s * gs:(s + 1) * gs, :])
    nc.compile()
    return nc


def kernel(x, Wf, bf, Wg, bg, Wh, bh, gamma):
    global _last_results, _last_nc
    x = np.asarray(x, dtype=np.float32)
    Wh = np.asarray(Wh, dtype=np.float32)
    bh = np.asarray(bh, dtype=np.float32)
    gam = np.float32(np.asarray(gamma))
    B, H, W, Cc = x.shape
    assert (B, H * W, Cc) == (NCORES, PIX, C), (B, H, W, Cc)

    whg = np.ascontiguousarray(gam * Wh, dtype=np.float32)
    bhg = (gam * bh).astype(np.float32)
    has_bias = bool(np.any(bhg != 0))

    nc = _build(has_bias)
    _last_nc = nc
    idn = np.eye(P, dtype=np.float32)
    xf = np.ascontiguousarray(x.reshape(B, PIX, Cc))
    in_maps = []
    for b in range(B):
        m = {"x": xf[b], "whg": whg, "idn": idn}
        if has_bias:
            m["ones"] = np.ones((1, P), np.float32)
            m["bhg"] = np.ascontiguousarray(bhg.reshape(1, C))
        in_maps.append(m)

    res = bass_utils.run_bass_kernel_spmd(nc, in_maps,
                                          core_ids=list(range(NCORES)))
    _last_results = res
    out = np.stack([res.results[b]["out"] for b in range(B)], axis=0)
    return out.reshape(B, H, W, Cc)


# revision 10
# speedup vs baseline: 1.2756x; 1.0059x over previous
"""Trainium2 Bass kernel for the CCSA (criss-cross self-attention) module.

The reference adds +INF_VAL (3.4e38, finite) on the H-axis diagonal of the
energy tensor before a joint softmax over the concatenated H+W axis.  In
float32 that makes the softmax an EXACT one-hot on the diagonal entry
(exp(small - 3.4e38) underflows to 0, exp(0) = 1), so att_h == I and
att_w == 0 identically, and the module collapses (bit-exactly, verified
against the jax reference) to:

    out = gamma * (x @ Wh + bh) + x

i.e. a residual 1x1 convolution.  The kernel below computes exactly that:
data-parallel over batch (one image per NeuronCore), per-core GEMM
[16384, 256] @ [256, 256] with the residual add fused in the epilogue.

Per-core pipeline (128-pixel chunks):
  - DMA a group of 2048 pixels [128, 16, 256] (p-major -> 16 KiB contiguous
    DRAM runs per partition)
  - PE-transpose each chunk's two 128-channel halves (C must sit on the
    partition axis for the contraction)
  - copy PSUM -> SBUF (ACT + DVE)
  - 2 accumulating matmuls (fp32r, moving operand Whg [128, 256])
  - DVE epilogue: out = psum + x (gamma folded into the weights host-side)
  - DMA the group back out
"""

import numpy as np

import concourse.bacc as bacc
import concourse.tile as tile
from concourse import mybir
from concourse import bass_utils

# Shapes fixed by the problem: x is [8, 128, 128, 256] float32.
NCORES = 8
P = 128            # SBUF partitions == pixels per chunk
C = 256            # channels
PIX = 128 * 128    # pixels per image
G = 16             # chunks per DMA group (2048 pixels, 2 MiB per transfer)
NGRP = PIX // (P * G)

F32 = mybir.dt.float32
F32R = mybir.dt.float32r
BF16 = mybir.dt.bfloat16
IDN_DT = F32R  # transpose-mode moving operand (walrus requires it match the
               # 32-bit data dtype; bf16 identity is rejected: NCC_IBIR034)

_last_results = None  # test.py reads exec_time_ns from here
_last_nc = None       # test.py runs TimelineSim on this


def _build(has_bias: bool):
    nc = bacc.Bacc("TRN2", target_bir_lowering=False, debug=False,
                   num_devices=NCORES)
    x_d = nc.dram_tensor("x", [PIX, C], F32R, kind="ExternalInput")
    whg_d = nc.dram_tensor("whg", [C, C], F32R, kind="ExternalInput")
    idn_d = nc.dram_tensor("idn", [P, P], IDN_DT, kind="ExternalInput")
    if has_bias:
        ones_d = nc.dram_tensor("ones", [1, P], F32R, kind="ExternalInput")
        bhg_d = nc.dram_tensor("bhg", [1, C], F32R, kind="ExternalInput")
    out_d = nc.dram_tensor("out", [PIX, C], F32, kind="ExternalOutput")

    # pixel index = n*(P*G) + p*G + g: each partition p owns G consecutive
    # pixels, so its DRAM run is G*C*4 = 16 KiB contiguous.
    xv = x_d.ap().rearrange("(n p g) c -> n p g c", n=NGRP, p=P, g=G)
    ov = out_d.ap().rearrange("(n p g) c -> n p g c", n=NGRP, p=P, g=G)

    LS = 2   # load pieces per group (1 MiB each)
    SS = 8   # store pieces per group (512 KiB each)
    with tile.TileContext(nc) as tc:
        with (
            tc.tile_pool(name="const", bufs=1) as cpool,
            tc.tile_pool(name="xin", bufs=3) as xin_pool,
            tc.tile_pool(name="xout", bufs=3) as xout_pool,
            tc.tile_pool(name="xt", bufs=3) as xt_pool,
            tc.tile_pool(name="pst", bufs=3, space="PSUM") as pst_pool,
            tc.tile_pool(name="pso", bufs=2, space="PSUM") as pso_pool,
        ):
            whg_sb = cpool.tile([P, 2, C], F32R)
            nc.sync.dma_start(whg_sb[:],
                              whg_d.ap().rearrange("(k p) c -> p k c", k=2))
            idn_sb = cpool.tile([P, P], IDN_DT)
            nc.sync.dma_start(idn_sb[:], idn_d.ap())
            if has_bias:
                ones_sb = cpool.tile([1, P], F32R)
                nc.sync.dma_start(ones_sb[:], ones_d.ap())
                bhg_sb = cpool.tile([1, C], F32R)
                nc.sync.dma_start(bhg_sb[:], bhg_d.ap())

            for n in range(NGRP):
                x_sb = xin_pool.tile([P, G, C], F32R, tag="xin")
                gl = G // LS
                for s in range(LS):
                    nc.sync.dma_start(x_sb[:, s * gl:(s + 1) * gl, :],
                                      xv[n, :, s * gl:(s + 1) * gl, :])
                o_sb = xout_pool.tile([P, G, C], F32, tag="xout")
                for g in range(G):
                    pst = pst_pool.tile([P, C], F32R, tag="pst")
                    nc.tensor.transpose(pst[:, 0:P], x_sb[:, g, 0:P], idn_sb[:])
                    nc.tensor.transpose(pst[:, P:C], x_sb[:, g, P:C], idn_sb[:])
                    xt = xt_pool.tile([P, C], F32R, tag="xt")
                    nc.scalar.copy(xt[:], pst[:])
                    pso = pso_pool.tile([P, C], F32, tag="pso")
                    nc.tensor.matmul(pso[:], xt[:, 0:P], whg_sb[:, 0, :],
                                     start=True, stop=False)
                    nc.tensor.matmul(pso[:], xt[:, P:C], whg_sb[:, 1, :],
                                     start=False, stop=not has_bias)
                    if has_bias:
                        nc.tensor.matmul(pso[:], ones_sb[:], bhg_sb[:],
                                         start=False, stop=True)
                    nc.vector.tensor_add(o_sb[:, g, :], pso[:], x_sb[:, g, :])
                gs = G // SS
                for s in range(SS):
                    nc.sync.dma_start(ov[n, :, s * gs:(s + 1) * gs, :],
                                      o_sb[:, s * gs:(s + 1) * gs, :])
    nc.compile()
    return nc


def kernel(x, Wf, bf, Wg, bg, Wh, bh, gamma):
    global _last_results, _last_nc
    x = np.asarray(x, dtype=np.float32)
    Wh = np.asarray(Wh, dtype=np.float32)
    bh = np.asarray(bh, dtype=np.float32)
    gam = np.float32(np.asarray(gamma))
    B, H, W, Cc = x.shape
    assert (B, H * W, Cc) == (NCORES, PIX, C), (B, H, W, Cc)

    whg = np.ascontiguousarray(gam * Wh, dtype=np.float32)
    bhg = (gam * bh).astype(np.float32)
    has_bias = bool(np.any(bhg != 0))

    nc = _build(has_bias)
    _last_nc = nc
    import ml_dtypes
    _idn_np = {BF16: ml_dtypes.bfloat16, F32: np.float32, F32R: np.float32}[IDN_DT]
    idn = np.eye(P, dtype=_idn_np)
    xf = np.ascontiguousarray(x.reshape(B, PIX, Cc))
    in_maps = []
    for b in range(B):
        m = {"x": xf[b], "whg": whg, "idn": idn}
        if has_bias:
            m["ones"] = np.ones((1, P), np.float32)
            m["bhg"] = np.ascontiguousarray(bhg.reshape(1, C))
        in_maps.append(m)

    res = bass_utils.run_bass_kernel_spmd(nc, in_maps,
                                          core_ids=list(range(NCORES)))
    _last_results = res
    out = np.stack([res.results[b]["out"] for b in range(B)], axis=0)
    return out.reshape(B, H, W, Cc)
